# revision 8
# baseline (speedup 1.0000x reference)
"""Trainium2 Bass kernel for nn_CombineEmbedder (GNN message passing).

Computation (N=262144 nodes, FEAT=64, D=128, DEPTH=2):
  x = layernorm(leaky2(leaky(raw@We)@Wl) + leaky(raw@We)) * 0.5        (embedder)
  2x: x += 0.25*rezero * leaky(0.5*(x[a]+x[b]) @ Wn + bn)              (message passing)
  out: (x, x@Ww, x@Wv)

Sharding: rows split across 8 NeuronCores. After the embedder and after
depth step 0, the per-core shard is AllGathered into a full replicated
[N,128] table so random-neighbor gathers (indirect DMA) are core-local.

Host-side folds: SlowNorm (mean/std) is folded into We/be. The 0.5
pair-mean is folded into Wn. The rezero step scale is applied as an
immediate in the fused update op.
"""
import os
import sys

for _p in ("/opt/trn_rl_repo", os.path.expanduser("~/.axon_site/_ro/trn_rl_repo")):
    if os.path.isdir(_p) and _p not in sys.path:
        sys.path.insert(0, _p)

import numpy as np

import concourse.bass as bass
import concourse.bacc as bacc
import concourse.mybir as mybir
import concourse.tile as tile
from concourse.masks import make_identity

P = 128
D = 128
FEAT = 64
N_FULL = 262144
N_CORES = 8
DEPTH = 2
SCALE_FEATURES = 0.5
SCALE_STEPS = (1.0 - SCALE_FEATURES) / DEPTH
LN_EPS = 1e-5
SLOPE = 0.01

f32 = mybir.dt.float32
i32 = mybir.dt.int32


def _leaky_from(nc, scr_ap, out_ap, in_ap, slope):
    """out = max(in, slope*in) == leaky_relu(in, slope).
    ACT computes slope*in (PSUM->SBUF scratch), DVE takes the max —
    each op reads at most one PSUM operand (BIR verifier requirement)."""
    nc.scalar.mul(out=scr_ap, in_=in_ap, mul=float(slope))
    nc.vector.tensor_tensor(out=out_ap, in0=scr_ap, in1=in_ap,
                            op=mybir.AluOpType.max)


def build_kernel(n_total, n_cores, step_scale, has_be, has_bl, has_bn):
    """Per-core SPMD program. n_total rows across n_cores cores."""
    ns = n_total // n_cores
    T = ns // P  # tiles per core
    nc = bacc.Bacc(None, target_bir_lowering=False, num_swdge_queues=4)

    raw_in = nc.declare_dram_parameter("raw", [ns, FEAT], f32, isOutput=False)
    ids_in = nc.declare_dram_parameter("ids", [ns, 2], i32, isOutput=False)
    we_in = nc.declare_dram_parameter("we", [FEAT, D], f32, isOutput=False)
    wl_in = nc.declare_dram_parameter("wl", [D, D], f32, isOutput=False)
    wn_in = nc.declare_dram_parameter("wn", [D, D], f32, isOutput=False)
    whd_in = nc.declare_dram_parameter("whd", [D, 2], f32, isOutput=False)
    bias_in = nc.declare_dram_parameter("biases", [3, D], f32, isOutput=False)
    x_out = nc.declare_dram_parameter("x_out", [ns, D], f32, isOutput=True)
    w_out = nc.declare_dram_parameter("w_out", [ns, 1], f32, isOutput=True)
    v_out = nc.declare_dram_parameter("v_out", [ns, 1], f32, isOutput=True)

    with tile.TileContext(nc) as tc:
        with (
            tc.tile_pool(name="resid", bufs=1) as resid,
            tc.tile_pool(name="dram", bufs=1, space="DRAM") as dram,
            tc.tile_pool(name="sb_e", bufs=3) as sb_e,
            tc.tile_pool(name="sb_s", bufs=4) as sb_s,
            tc.tile_pool(name="sm", bufs=3) as sm,
            tc.tile_pool(name="ps", bufs=2, space="PSUM") as ps,
        ):
            # ---- resident constants ----
            ids_sb = resid.tile([P, T * 2], i32)
            nc.sync.dma_start(out=ids_sb[:].rearrange("p (t k) -> p t k", k=2),
                              in_=ids_in.ap().rearrange("(t p) k -> p t k", p=P))
            we_sb = resid.tile([FEAT, D], f32)
            nc.sync.dma_start(out=we_sb[:], in_=we_in[:, :])
            wl_sb = resid.tile([D, D], f32)
            nc.sync.dma_start(out=wl_sb[:], in_=wl_in[:, :])
            wn_sb = resid.tile([D, D], f32)
            nc.sync.dma_start(out=wn_sb[:], in_=wn_in[:, :])
            whd_sb = resid.tile([D, 2], f32)
            nc.sync.dma_start(out=whd_sb[:], in_=whd_in[:, :])
            # identity (cols 0..127) with an extra all-ones col 128 for row sums
            id1_sb = resid.tile([P, P + 1], f32)
            make_identity(nc, id1_sb[:, :P])
            nc.vector.memset(id1_sb[:, P:P + 1], 1.0 / D)
            be_row = resid.tile([1, D], f32)
            nc.sync.dma_start(out=be_row[:], in_=bias_in[0:1, :])
            bl_row = resid.tile([1, D], f32)
            nc.sync.dma_start(out=bl_row[:], in_=bias_in[1:2, :])
            bn_row = resid.tile([1, D], f32)
            nc.sync.dma_start(out=bn_row[:], in_=bias_in[2:3, :])
            ones_row = resid.tile([1, P], f32)
            nc.vector.memset(ones_row[:], 1.0)
            eps4_sb = resid.tile([P, 1], f32)
            nc.vector.memset(eps4_sb[:], 4.0 * LN_EPS)
            wvT_stage = resid.tile([2, ns], f32)

            # ---- DRAM internals ----
            ag0 = dram.tile([ns, D], f32)
            ag1 = dram.tile([ns, D], f32)
            tbl0 = dram.tile([n_total, D], f32, addr_space="Shared")
            tbl1 = dram.tile([n_total, D], f32, addr_space="Shared")

            # ================= Phase E: embedder =================
            for t in range(T):
                raw_t = sb_e.tile([P, FEAT], f32, tag="raw_t")
                nc.sync.dma_start(out=raw_t[:], in_=raw_in[t * P:(t + 1) * P, :])
                p_rT = ps.tile([FEAT, P], f32, tag="psA", space="PSUM")
                nc.tensor.transpose(out=p_rT[:], in_=raw_t[:], identity=id1_sb[:, :P])
                rawT = sb_e.tile([FEAT, P], f32, tag="rawT")
                nc.scalar.copy(out=rawT[:], in_=p_rT[:])

                p_x1T = ps.tile([D, P], f32, tag="psB", space="PSUM")
                nc.tensor.matmul(out=p_x1T[:], lhsT=we_sb[:], rhs=rawT[:],
                                 start=True, stop=not has_be)
                if has_be:
                    nc.tensor.matmul(out=p_x1T[:], lhsT=be_row[:], rhs=ones_row[:],
                                     start=False, stop=True)
                l1s = sb_e.tile([D, P], f32, tag="l1s")
                x1T = sb_e.tile([D, P], f32, tag="x1T")
                _leaky_from(nc, l1s[:], x1T[:], p_x1T[:], SLOPE)

                p_hT = ps.tile([D, P], f32, tag="psC", space="PSUM")
                nc.tensor.matmul(out=p_hT[:], lhsT=wl_sb[:], rhs=x1T[:],
                                 start=True, stop=not has_bl)
                if has_bl:
                    nc.tensor.matmul(out=p_hT[:], lhsT=bl_row[:], rhs=ones_row[:],
                                     start=False, stop=True)
                l2s = sb_e.tile([D, P], f32, tag="l2s")
                hT2 = sb_e.tile([D, P], f32, tag="hT2")
                _leaky_from(nc, l2s[:], hT2[:], p_hT[:], SLOPE * SLOPE)
                x2T = sb_e.tile([D, P], f32, tag="x2T")
                nc.vector.tensor_add(out=x2T[:], in0=hT2[:], in1=x1T[:])

                # x2 (node-major) + row sums via matmul with [I | 1]
                p_x2s = ps.tile([P, P + 1], f32, tag="psD", space="PSUM")
                nc.tensor.matmul(out=p_x2s[:], lhsT=x2T[:], rhs=id1_sb[:, :],
                                 start=True, stop=True)

                # col P of p_x2s is x2 @ (1/D) = mu
                mu_sb = sm.tile([P, 1], f32, tag="mu_sb")
                nc.vector.tensor_copy(out=mu_sb[:], in_=p_x2s[:, P:P + 1])
                sq_scr = sb_e.tile([P, P], f32, tag="sq_scr")
                sumsq = sm.tile([P, 1], f32, tag="sumsq")
                nc.scalar.activation(out=sq_scr[:], in_=p_x2s[:, :P],
                                     func=mybir.ActivationFunctionType.Square,
                                     accum_out=sumsq[:])
                mu2n = sm.tile([P, 1], f32, tag="mu2n")
                nc.vector.tensor_scalar(out=mu2n[:], in0=mu_sb[:],
                                        scalar1=mu_sb[:], scalar2=-4.0,
                                        op0=mybir.AluOpType.mult,
                                        op1=mybir.AluOpType.mult)
                var4 = sm.tile([P, 1], f32, tag="var4")
                nc.vector.scalar_tensor_tensor(out=var4[:], in0=sumsq[:],
                                               scalar=4.0 / D, in1=mu2n[:],
                                               op0=mybir.AluOpType.mult,
                                               op1=mybir.AluOpType.add)
                std2 = sm.tile([P, 1], f32, tag="std2")
                # sqrt(var4 + 4*eps) = 2*sqrt(var+eps)
                nc.scalar.activation(out=std2[:], in_=var4[:],
                                     func=mybir.ActivationFunctionType.Sqrt,
                                     bias=eps4_sb[:])
                rstd = sm.tile([P, 1], f32, tag="rstd")
                nc.vector.reciprocal(out=rstd[:], in_=std2[:])  # 0.5/sqrt(var+eps)
                x0 = sb_e.tile([P, P], f32, tag="x0")
                nc.vector.tensor_scalar(out=x0[:], in0=p_x2s[:, :P],
                                        scalar1=mu_sb[:], scalar2=rstd[:],
                                        op0=mybir.AluOpType.subtract,
                                        op1=mybir.AluOpType.mult)
                nc.sync.dma_start(out=ag0[t * P:(t + 1) * P, :], in_=x0[:])

            # ================= AllGather 0 =================
            nc.gpsimd.collective_compute(
                "AllGather", mybir.AluOpType.bypass,
                replica_groups=[list(range(n_cores))],
                ins=[ag0.opt()], outs=[tbl0.opt()])

            # ================= depth steps =================
            for step in range(DEPTH):
                tbl_src = tbl0 if step == 0 else tbl1
                xin = ag0 if step == 0 else ag1
                last = step == DEPTH - 1
                for t in range(T):
                    ga = sb_s.tile([P, D], f32, tag="ga")
                    nc.gpsimd.indirect_dma_start(
                        out=ga[:], out_offset=None, in_=tbl_src[:],
                        in_offset=bass.IndirectOffsetOnAxis(
                            ap=ids_sb[:, 2 * t:2 * t + 1], axis=0))
                    gb = sb_s.tile([P, D], f32, tag="gb")
                    nc.gpsimd.indirect_dma_start(
                        out=gb[:], out_offset=None, in_=tbl_src[:],
                        in_offset=bass.IndirectOffsetOnAxis(
                            ap=ids_sb[:, 2 * t + 1:2 * t + 2], axis=0))
                    msum = sb_s.tile([P, D], f32, tag="msum")
                    nc.vector.tensor_add(out=msum[:], in0=ga[:], in1=gb[:])

                    x_t = sb_s.tile([P, D], f32, tag="x_t")
                    nc.sync.dma_start(out=x_t[:], in_=xin[t * P:(t + 1) * P, :])

                    p_mT = ps.tile([D, P], f32, tag="psA", space="PSUM")
                    nc.tensor.transpose(out=p_mT[:], in_=msum[:], identity=id1_sb[:, :P])
                    mT = sb_s.tile([D, P], f32, tag="mT")
                    nc.scalar.copy(out=mT[:], in_=p_mT[:])

                    p_upd = ps.tile([P, D], f32, tag="psB", space="PSUM")
                    nc.tensor.matmul(out=p_upd[:], lhsT=mT[:], rhs=wn_sb[:],
                                     start=True, stop=not has_bn)
                    if has_bn:
                        nc.tensor.matmul(out=p_upd[:], lhsT=ones_row[:],
                                         rhs=bn_row[:], start=False, stop=True)
                    lus = sb_s.tile([P, D], f32, tag="lus")
                    upd = sb_s.tile([P, D], f32, tag="upd")
                    _leaky_from(nc, lus[:], upd[:], p_upd[:], SLOPE)
                    new = sb_s.tile([P, D], f32, tag="new")
                    nc.vector.scalar_tensor_tensor(
                        out=new[:], in0=upd[:], scalar=float(step_scale), in1=x_t[:],
                        op0=mybir.AluOpType.mult, op1=mybir.AluOpType.add)
                    if not last:
                        nc.sync.dma_start(out=ag1[t * P:(t + 1) * P, :], in_=new[:])
                    else:
                        nc.sync.dma_start(out=x_out[t * P:(t + 1) * P, :], in_=new[:])
                        p_nT = ps.tile([D, P], f32, tag="psC", space="PSUM")
                        nc.tensor.transpose(out=p_nT[:], in_=new[:],
                                            identity=id1_sb[:, :P])
                        nT = sb_s.tile([D, P], f32, tag="nT")
                        nc.scalar.copy(out=nT[:], in_=p_nT[:])
                        p_hv = ps.tile([2, P], f32, tag="psD", space="PSUM")
                        nc.tensor.matmul(out=p_hv[:], lhsT=whd_sb[:], rhs=nT[:],
                                         start=True, stop=True)
                        nc.vector.tensor_copy(
                            out=wvT_stage[:, t * P:(t + 1) * P], in_=p_hv[:])
                if not last:
                    nc.gpsimd.collective_compute(
                        "AllGather", mybir.AluOpType.bypass,
                        replica_groups=[list(range(n_cores))],
                        ins=[ag1.opt()], outs=[tbl1.opt()])

            nc.sync.dma_start(out=w_out.ap().rearrange("n 1 -> 1 n"),
                              in_=wvT_stage[0:1, :])
            nc.sync.dma_start(out=v_out.ap().rearrange("n 1 -> 1 n"),
                              in_=wvT_stage[1:2, :])
    nc.finalize()
    # Spread the indirect gathers round-robin over 4 SWDGE queues (small win;
    # the Q7 descriptor generator is the real bottleneck). Safe post-finalize:
    # Tile gives each SW-DMA its own completion sem, so consumers don't rely
    # on single-queue FIFO order.
    qi = 0
    for bb in nc.main_func.blocks:
        for ins in bb.instructions:
            if (isinstance(ins, mybir.InstDMACopy)
                    and getattr(ins, "queue", None) == "qPoolDynamic"):
                q = qi % 4
                if q:
                    ins.queue = f"qPoolDynamic{q}"
                qi += 1
    return nc


def host_prep(raw_feats, id_map, slow_mean, slow_std, W_embd, b_embd, Wl, bl,
              W_node, b_node, rezero, W_w, W_v):
    """Fold SlowNorm/means into weights; return per-core input maps + flags."""
    raw_feats = np.ascontiguousarray(np.asarray(raw_feats, np.float32))
    ids = np.ascontiguousarray(np.asarray(id_map)[:, 0, :].astype(np.int32))
    inv = 1.0 / (np.asarray(slow_std, np.float64) + 0.001)
    W_embd64 = np.asarray(W_embd, np.float64)
    we = (W_embd64 * inv[:, None]).astype(np.float32)
    be = (np.asarray(b_embd, np.float64)
          - (np.asarray(slow_mean, np.float64) * inv) @ W_embd64).astype(np.float32)
    wl0 = np.ascontiguousarray(np.asarray(Wl, np.float32)[0])
    bl0 = np.asarray(bl, np.float32)[0]
    wn = np.ascontiguousarray(0.5 * np.asarray(W_node, np.float32))
    bn = np.asarray(b_node, np.float32)
    whd = np.ascontiguousarray(
        np.concatenate([np.asarray(W_w, np.float32),
                        np.asarray(W_v, np.float32)], axis=1))
    step_scale = float(SCALE_STEPS * np.asarray(rezero, np.float64).ravel()[0])
    biases = np.stack([be, bl0, bn]).astype(np.float32)
    flags = (bool(np.any(be != 0)), bool(np.any(bl0 != 0)), bool(np.any(bn != 0)))

    n_total = raw_feats.shape[0]
    ns = n_total // N_CORES
    in_maps = []
    for c in range(N_CORES):
        in_maps.append({
            "raw": raw_feats[c * ns:(c + 1) * ns],
            "ids": ids[c * ns:(c + 1) * ns],
            "we": we, "wl": wl0, "wn": wn, "whd": whd, "biases": biases,
        })
    return in_maps, step_scale, flags


_RUNNER_CACHE = {}


class _SpmdRunner:
    """Compile the SPMD program once via bass2jax/PJRT; reuse across calls."""

    def __init__(self, nc, n_cores):
        import jax
        from jax.sharding import Mesh, PartitionSpec
        from jax.experimental.shard_map import shard_map
        from concourse.bass2jax import (_bass_exec_p, install_neuronx_cc_hook,
                                        partition_id_tensor)
        install_neuronx_cc_hook()
        self.n_cores = n_cores
        self._jax = jax
        partition_name = nc.partition_id_tensor.name if nc.partition_id_tensor else None
        in_names, out_names, out_avals, zero_outs = [], [], [], []
        for alloc in nc.m.functions[0].allocations:
            if not isinstance(alloc, mybir.MemoryLocationSet):
                continue
            name = alloc.memorylocations[0].name
            if alloc.kind == "ExternalInput":
                if name != partition_name:
                    in_names.append(name)
            elif alloc.kind == "ExternalOutput":
                out_names.append(name)
                shape = tuple(alloc.tensor_shape)
                dtype = mybir.dt.np(alloc.dtype)
                out_avals.append(jax.core.ShapedArray(shape, dtype))
                zero_outs.append(np.zeros(shape, dtype))
        self.in_names, self.out_names = in_names, out_names
        self.out_avals, self.zero_outs = out_avals, zero_outs
        n_params, n_outs = len(in_names), len(out_avals)
        self.n_params = n_params
        all_in = list(in_names) + list(out_names)
        if partition_name is not None:
            all_in.append(partition_name)
        donate = tuple(range(n_params, n_params + n_outs))

        def _body(*args):
            operands = list(args)
            if partition_name is not None:
                operands.append(partition_id_tensor())
            return tuple(_bass_exec_p.bind(
                *operands, out_avals=tuple(out_avals), in_names=tuple(all_in),
                out_names=tuple(out_names), lowering_input_output_aliases=(),
                sim_require_finite=True, sim_require_nnan=True, nc=nc))

        devices = jax.devices()[:n_cores]
        assert len(devices) >= n_cores or len(devices) == n_cores, devices
        mesh = Mesh(np.asarray(devices), ("core",))
        in_specs = (PartitionSpec("core"),) * (n_params + n_outs)
        out_specs = (PartitionSpec("core"),) * len(out_names)
        self.fn = jax.jit(
            shard_map(_body, mesh=mesh, in_specs=in_specs, out_specs=out_specs,
                      check_rep=False),
            donate_argnums=donate, keep_unused=True)

    def __call__(self, in_maps):
        per_core = [[np.asarray(m[n]) for n in self.in_names] for m in in_maps]
        concat_in = [np.concatenate([per_core[c][i] for c in range(self.n_cores)], axis=0)
                     for i in range(self.n_params)]
        concat_zeros = [np.zeros((self.n_cores * z.shape[0], *z.shape[1:]), z.dtype)
                        for z in self.zero_outs]
        out_arrs = self.fn(*concat_in, *concat_zeros)
        self._jax.block_until_ready(out_arrs)
        return [
            {n: np.asarray(out_arrs[i]).reshape(self.n_cores, *self.out_avals[i].shape)[c]
             for i, n in enumerate(self.out_names)}
            for c in range(self.n_cores)
        ]


def _get_runner(n_total, step_scale, flags):
    key = (n_total, float(step_scale), flags)
    if key not in _RUNNER_CACHE:
        nc = build_kernel(n_total, N_CORES, step_scale, *flags)
        _RUNNER_CACHE[key] = _SpmdRunner(nc, N_CORES)
    return _RUNNER_CACHE[key]


def kernel(raw_feats, uids, id_map, slow_mean, slow_std, W_embd, b_embd,
           Wl, bl, W_node, b_node, rezero, W_w, W_v):
    in_maps, step_scale, flags = host_prep(
        raw_feats, id_map, slow_mean, slow_std, W_embd, b_embd, Wl, bl,
        W_node, b_node, rezero, W_w, W_v)
    n_total = np.asarray(raw_feats).shape[0]
    runner = _get_runner(n_total, step_scale, flags)
    res = runner(in_maps)
    x = np.concatenate([res[c]["x_out"] for c in range(N_CORES)], axis=0)
    w = np.concatenate([res[c]["w_out"] for c in range(N_CORES)], axis=0)
    v = np.concatenate([res[c]["v_out"] for c in range(N_CORES)], axis=0)
    return x, w, v
